# revision 10
# baseline (speedup 1.0000x reference)
"""Trainium2 Bass kernel for group-quant (fake int8, V=64) + Linear.

reference math (per row of x):
    absmax over feature-groups of 64 -> delta = max(2*absmax/254, 1e-5)
    xq = clip(round(x/delta), -127, 127) * delta      (fake quant)
    out = xq @ W.T + b

Sharding: data-parallel on tokens across 8 cores (1024 rows each);
W (pre-transposed/packed fp16 on host) + b (fp16) replicated.

Device schedule per core (v2 — W-half resident, t-tile pipeline):
  quant runs per 128-token tile in natural layout, column-split across
  the vector and gpsimd engines (group absmax reduce, exact RNE round
  via the +/-1.5*2^23 trick, dequant to fp16), then ONE whole-tile
  SBUF->SBUF XBAR transpose (scalar queue, isolated from copy DMAs)
  produces x~^T [128k, KT, 128t].  Matmuls run in two phases, each with
  half of W^T (4 oc-chunks of 512, 16.8MB fp16) resident in SBUF:
  per t-tile, k-outer/oc-inner so 4 consecutive matmuls share one
  stationary (LDWEIGHTS amortized 4x), accumulating into 4 PSUM banks,
  ping-ponged with the other 4 across t-tiles.  First/last tiles of
  each phase run oc-major to stagger W DMA arrival/swap.  x~^T tiles
  are spilled to DRAM in phase A and re-loaded in phase B.  PSUM is
  evacuated fused with the bias add on vector; output DMAs ride the
  vector queue so they never head-of-line-block W loads (sync queue).
"""

import numpy as np

import concourse.bass as bass
import concourse.mybir as mybir
import concourse.tile as tile
from concourse.bass_utils import run_bass_kernel_spmd

N_CORES = 8
MAGIC = 1.5 * 2.0**23      # fp32 round-to-nearest-even constant
QSCALE = 1.0 / 127.0       # 2/(qmax-qmin) with qmax=127, qmin=-127
DELTA_MIN = 1e-5


def _split_multiwait(nc):
    """This walrus build allows at most ONE sync wait per instruction
    ("Too many sync wait commands", CoreV3GenImpl setupSyncWait) and none
    on Drain. Tile freely attaches several waits to one instruction, so
    post-process: move excess waits onto single-wait NoOps inserted just
    before the instruction on the same engine queue (semantics identical —
    the queue stalls at the nop instead of at the instruction)."""
    nid = 0
    for fn in nc.m.functions:
        for bb in fn.blocks:
            insts = list(bb.instructions)
            out = []
            changed = False
            for inst in insts:
                si = inst.sync_info
                waits = list(si.on_wait) if si is not None and si.on_wait else []
                limit = 0 if type(inst).__name__ == "InstDrain" else 1
                if len(waits) > limit:
                    changed = True
                    keep = waits[len(waits) - limit :] if limit else []
                    for w in waits[: len(waits) - limit]:
                        nid += 1
                        out.append(
                            mybir.InstNoOp(
                                name=f"WSPLIT-{nid}",
                                engine=inst.engine,
                                bass_nofuse=True,
                                ins=[],
                                outs=[],
                                sync_info=mybir.SyncInfo(on_wait=[w], on_update=[]),
                            )
                        )
                    si.on_wait = keep
                out.append(inst)
            if changed:
                try:
                    bb.instructions = out
                except Exception:
                    bb.instructions[:] = out


def build(T=1024, K=4096, O=4096, V=64, CV=1280, wq_split=4, split=True,
          noload=False):
    f32, f16 = mybir.dt.float32, mybir.dt.float16
    P = 128
    G = K // V                 # quant groups per row (64)
    KT = K // P                # contraction tiles (32)
    NT = T // P                # token tiles per core (8)
    OC = 512                   # oc chunk (psum bank width fp32)
    NOC = O // OC              # 8
    NPH = 2                    # W halves
    OCPH = NOC // NPH          # oc chunks per phase (4)
    GV = CV // V               # vector-side quant groups
    KQ = KT // wq_split        # W load quarters

    nc = bass.Bass()
    x = nc.dram_tensor("x", [T, K], f32, kind="ExternalInput")
    wt = nc.dram_tensor("wt", [NOC, P, KT * OC], f16, kind="ExternalInput")
    bvec = nc.dram_tensor("b", [O], f16, kind="ExternalInput")
    out = nc.dram_tensor("out", [T, O], f32, kind="ExternalOutput")
    xtd = nc.dram_tensor("xtd", [NT, P, KT * P], f16)  # x~^T spill

    mult = mybir.AluOpType.mult
    add = mybir.AluOpType.add
    sub = mybir.AluOpType.subtract
    amax_op = mybir.AluOpType.max

    with tile.TileContext(nc) as tc:
        with (
            tc.tile_pool(name="x", bufs=2) as pool_x,
            tc.tile_pool(name="xh", bufs=2) as pool_xh,
            tc.tile_pool(name="st", bufs=2) as pool_s,
            tc.tile_pool(name="xt", bufs=2) as pool_xt,
            tc.tile_pool(name="w", bufs=1) as pool_w,
            tc.tile_pool(name="bias", bufs=1) as pool_b,
            tc.tile_pool(name="o", bufs=3) as pool_o,
            tc.tile_pool(name="ps", bufs=1, space="PSUM") as pool_ps,
        ):
            def post_bias(ph):
                bt = pool_b.tile([P, OCPH, OC], f16, tag="b", name=f"b{ph}")
                bsl = bvec[ph * OCPH * OC : (ph + 1) * OCPH * OC]
                bb = bass.AP(
                    tensor=bsl.tensor, offset=bsl.offset, ap=[[0, P], *bsl.ap]
                )
                nc.sync.dma_start(out=bt.rearrange("p c o -> p (c o)"), in_=bb)
                return bt

            def post_w(ph, oc):
                wtile = pool_w.tile(
                    [P, KT, OC], f16, tag=f"w{oc}", name=f"w{ph}_{oc}"
                )
                g = ph * OCPH + oc
                for q in range(wq_split):
                    nc.sync.dma_start(
                        out=wtile[:, q * KQ : (q + 1) * KQ, :].rearrange(
                            "p k o -> p (k o)"
                        ),
                        in_=wt[g][:, q * KQ * OC : (q + 1) * KQ * OC],
                    )
                return wtile

            def emit_quant(t):
                # halves H0=[0:K/2], H1=[K/2:K]; vector owns groups [0:GV]
                # (inside H0), gpsimd the rest; per-half round lets the
                # first XBAR transpose (and first matmul) start early.
                H = K // 2
                GH = G // 2
                xt_ = pool_x.tile([P, K], f32, tag="x", name=f"x{t}")
                nc.gpsimd.dma_start(out=xt_[:, :H], in_=x[t * P : (t + 1) * P, :H])
                nc.gpsimd.dma_start(out=xt_[:, H:], in_=x[t * P : (t + 1) * P, H:])
                xr = xt_.rearrange("p (g v) -> p g v", v=V)
                # gpsimd can only reduce along partitions -> reduce on vector
                amax = pool_s.tile([P, G], f32, tag="amax", name=f"amax{t}")
                nc.vector.tensor_reduce(
                    out=amax[:, :GH], in_=xr[:, :GH, :], axis=mybir.AxisListType.X,
                    op=amax_op, apply_absolute_value=True,
                )
                nc.vector.tensor_reduce(
                    out=amax[:, GH:], in_=xr[:, GH:, :], axis=mybir.AxisListType.X,
                    op=amax_op, apply_absolute_value=True,
                )
                delta = pool_s.tile([P, G], f32, tag="delta", name=f"delta{t}")
                nc.vector.tensor_scalar(
                    out=delta[:], in0=amax[:],
                    scalar1=QSCALE, scalar2=DELTA_MIN, op0=mult, op1=amax_op,
                )
                recip = pool_s.tile([P, G], f32, tag="recip", name=f"recip{t}")
                nc.vector.reciprocal(out=recip[:], in_=delta[:])

                def rmul(eng, g0, g1):   # x/delta on group range
                    eng.tensor_tensor(
                        out=xr[:, g0:g1, :], in0=xr[:, g0:g1, :],
                        in1=recip[:, g0:g1, None].to_broadcast((P, g1 - g0, V)),
                        op=mult,
                    )

                def dmul(eng, g0, g1):   # dequant (fp16 out) on group range
                    eng.tensor_tensor(
                        out=xhr[:, g0:g1, :], in0=xr[:, g0:g1, :],
                        in1=delta[:, g0:g1, None].to_broadcast((P, g1 - g0, V)),
                        op=mult,
                    )

                xh_t = pool_xh.tile([P, K], f16, tag="xh", name=f"xh{t}")
                xhr = xh_t.rearrange("p (g v) -> p g v", v=V)
                rmul(nc.vector, 0, GV)
                rmul(nc.gpsimd, GV, GH)
                rmul(nc.gpsimd, GH, G)
                # exact fp32 round-to-nearest-even; |x/delta| <= 127 < 2^22
                # (on vector: Pool's fused 2-op tensor_scalar is ~15 ns/elem
                # vs DVE's ~0.6)
                nc.vector.tensor_scalar(
                    out=xt_[:, :H], in0=xt_[:, :H],
                    scalar1=MAGIC, scalar2=MAGIC, op0=add, op1=sub,
                )
                dmul(nc.vector, 0, GV)
                dmul(nc.gpsimd, GV, GH)
                nc.vector.tensor_scalar(
                    out=xt_[:, H:], in0=xt_[:, H:],
                    scalar1=MAGIC, scalar2=MAGIC, op0=add, op1=sub,
                )
                dmul(nc.gpsimd, GH, G)
                # per-half XBAR transpose -> [128k, KT, 128t], then spill
                xts_t = pool_xt.tile([P, KT, P], f16, tag="xt", name=f"xts{t}")
                nc.scalar.dma_start_transpose(xts_t[:, : KT // 2, :], xh_t[:, :H])
                nc.scalar.dma_start_transpose(xts_t[:, KT // 2 :, :], xh_t[:, H:])
                nc.sync.dma_start(
                    out=xtd[t], in_=xts_t.rearrange("p k q -> p (k q)")
                )
                return xts_t

            def evac(ph, t, oc, ps, bt):
                ot = pool_o.tile([P, OC], f32, tag="o", name=f"ot{ph}_{t}_{oc}")
                nc.vector.tensor_tensor(out=ot[:], in0=ps[:], in1=bt[:, oc, :], op=add)
                g = ph * OCPH + oc
                nc.gpsimd.dma_start(
                    out=out[t * P : (t + 1) * P, g * OC : (g + 1) * OC], in_=ot[:]
                )

            def mm(ps, lhsT, rhs, start, stop, first):
                inst = nc.tensor.matmul(ps, lhsT, rhs, start=start, stop=stop)
                if noload and not first:
                    try:
                        inst.ldweights = False
                    except Exception:
                        try:
                            inst.ins.ldweights = False
                        except Exception:
                            pass

            def emit_mm(ph, t, xts_t, bt, ocs, mode, after_group=None):
                if mode == "oc":
                    for oc in ocs:
                        ps = pool_ps.tile([P, OC], f32, tag=f"ps{t % 2}_{oc}",
                                          name=f"ps{ph}_{t}_{oc}")
                        for kt in range(KT):
                            mm(ps[:], xts_t[:, kt, :], wcur[oc][:, kt, :],
                               kt == 0, kt == KT - 1, True)
                        evac(ph, t, oc, ps, bt)
                        if after_group is not None:
                            after_group(oc)
                else:
                    pss = {
                        oc: pool_ps.tile([P, OC], f32, tag=f"ps{t % 2}_{oc}",
                                         name=f"ps{ph}_{t}_{oc}")
                        for oc in ocs
                    }
                    for kt in range(KT):
                        for oc in ocs:
                            mm(pss[oc][:], xts_t[:, kt, :], wcur[oc][:, kt, :],
                               kt == 0, kt == KT - 1, oc == ocs[0])
                    for oc in ocs:
                        evac(ph, t, oc, pss[oc], bt)

            # ---- phase A (oc 0..3 resident), quant interleaved ----
            btA = post_bias(0)
            wcur = [post_w(0, oc) for oc in range(OCPH)]

            def swapcb(oc):
                wcur[oc] = post_w(1, oc)

            # Segments stagger t0's oc-groups around t1 so the W stream has
            # time to arrive (phase A) / swap in (phase B) without stalling
            # the PE, and the last phase-A tile runs oc-major so each W-half
            # buffer frees (and reloads with phase B) one group at a time.
            def segments(ph):
                segs = [(0, [0, 1], "oc", None), (1, ALL, "k", None),
                        (0, [2, 3], "oc", None)]
                for t in range(2, NT - 1):
                    segs.append((t, ALL, "k", None))
                segs.append((NT - 1, ALL, "oc" if ph == 0 else "k",
                             swapcb if ph == 0 else None))
                return segs

            ALL = list(range(OCPH))

            # ---- phase A (oc 0..3 resident), quant interleaved ----
            tiles = {}
            segsA = segments(0)
            qnext = 0

            def emit_q_upto(n):
                nonlocal qnext
                while qnext < min(n, NT):
                    tiles[qnext] = emit_quant(qnext)
                    qnext += 1

            emit_q_upto(2)
            for si, (t, ocs, mode, cb) in enumerate(segsA):
                emit_mm(0, t, tiles[t], btA, ocs, mode, cb)
                emit_q_upto(3 + si)

            # ---- phase B (oc 4..7 resident), x~^T re-loaded from DRAM ----
            def reload(t):
                xr_t = pool_xt.tile([P, KT, P], f16, tag="xt", name=f"xtr{t}")
                nc.sync.dma_start(
                    out=xr_t.rearrange("p k q -> p (k q)"), in_=xtd[t]
                )
                return xr_t

            rel = {0: reload(0), 1: reload(1)}
            btB = post_bias(1)
            rnext = 2

            for t, ocs, mode, cb in segments(1):
                emit_mm(1, t, rel[t], btB, ocs, mode, cb)
                if rnext < NT:
                    rel[rnext] = reload(rnext)
                    rnext += 1

    if split:
        _split_multiwait(nc)
    return nc


_CACHED = {}

# test-harness knobs (kernel() defaults are what the grader uses)
TRACE = False
LAST_RESULT = None
BUILD_KW = {}


def _get_nc(shape_key):
    if shape_key not in _CACHED:
        T, K, O = shape_key
        _CACHED[shape_key] = build(T=T, K=K, O=O, **BUILD_KW)
    return _CACHED[shape_key]


def pack_w(W: np.ndarray, OC: int = 512, P: int = 128) -> np.ndarray:
    # [out,in] -> W^T [in,out] fp16, packed [NOC, P, KT*OC] so each per-core
    # o-chunk W load is one fully contiguous DMA
    K, O = W.shape[1], W.shape[0]
    KT, NOC = K // P, O // OC
    wt = np.ascontiguousarray(W.T).astype(np.float16)         # [K, O]
    z = wt.reshape(KT, P, NOC, OC).transpose(2, 1, 0, 3)      # [NOC, P, KT, OC]
    return np.ascontiguousarray(z.reshape(NOC, P, KT * OC))


def kernel(x: np.ndarray, W: np.ndarray, b: np.ndarray) -> np.ndarray:
    global LAST_RESULT
    n, k = x.shape               # 8192, 4096
    o = W.shape[0]               # 4096
    assert n % N_CORES == 0
    tpc = n // N_CORES
    nc = _get_nc((tpc, k, o))

    wt = pack_w(W)
    b16 = np.ascontiguousarray(b.astype(np.float16))
    xs = np.ascontiguousarray(x.astype(np.float32)).reshape(N_CORES, tpc, k)
    in_maps = [{"x": xs[i], "wt": wt, "b": b16} for i in range(N_CORES)]
    res = run_bass_kernel_spmd(nc, in_maps, list(range(N_CORES)), trace=TRACE)
    LAST_RESULT = res
    return np.concatenate([res.results[i]["out"] for i in range(N_CORES)], axis=0)


# revision 13
# speedup vs baseline: 1.0281x; 1.0281x over previous
"""Trainium2 Bass kernel for group-quant (fake int8, V=64) + Linear.

reference math (per row of x):
    absmax over feature-groups of 64 -> delta = max(2*absmax/254, 1e-5)
    xq = clip(round(x/delta), -127, 127) * delta      (fake quant)
    out = xq @ W.T + b

Sharding: data-parallel on tokens across 8 cores (1024 rows each);
W (pre-transposed/packed fp16 on host) + b (fp16) replicated.

Device schedule per core (v2 — W-half resident, t-tile pipeline):
  quant runs per 128-token tile in natural layout, column-split across
  the vector and gpsimd engines (group absmax reduce, exact RNE round
  via the +/-1.5*2^23 trick, dequant to fp16), then ONE whole-tile
  SBUF->SBUF XBAR transpose (scalar queue, isolated from copy DMAs)
  produces x~^T [128k, KT, 128t].  Matmuls run in two phases, each with
  half of W^T (4 oc-chunks of 512, 16.8MB fp16) resident in SBUF:
  per t-tile, k-outer/oc-inner so 4 consecutive matmuls share one
  stationary (LDWEIGHTS amortized 4x), accumulating into 4 PSUM banks,
  ping-ponged with the other 4 across t-tiles.  First/last tiles of
  each phase run oc-major to stagger W DMA arrival/swap.  x~^T tiles
  are spilled to DRAM in phase A and re-loaded in phase B.  PSUM is
  evacuated fused with the bias add on vector; output DMAs ride the
  vector queue so they never head-of-line-block W loads (sync queue).
"""

import numpy as np

import concourse.bass as bass
import concourse.mybir as mybir
import concourse.tile as tile
from concourse.bass_utils import run_bass_kernel_spmd

N_CORES = 8
MAGIC = 1.5 * 2.0**23      # fp32 round-to-nearest-even constant
QSCALE = 1.0 / 127.0       # 2/(qmax-qmin) with qmax=127, qmin=-127
DELTA_MIN = 1e-5


def _split_multiwait(nc):
    """This walrus build allows at most ONE sync wait per instruction
    ("Too many sync wait commands", CoreV3GenImpl setupSyncWait) and none
    on Drain. Tile freely attaches several waits to one instruction, so
    post-process: move excess waits onto single-wait NoOps inserted just
    before the instruction on the same engine queue (semantics identical —
    the queue stalls at the nop instead of at the instruction)."""
    nid = 0
    for fn in nc.m.functions:
        for bb in fn.blocks:
            insts = list(bb.instructions)
            out = []
            changed = False
            for inst in insts:
                si = inst.sync_info
                waits = list(si.on_wait) if si is not None and si.on_wait else []
                limit = 0 if type(inst).__name__ == "InstDrain" else 1
                if len(waits) > limit:
                    changed = True
                    keep = waits[len(waits) - limit :] if limit else []
                    for w in waits[: len(waits) - limit]:
                        nid += 1
                        out.append(
                            mybir.InstNoOp(
                                name=f"WSPLIT-{nid}",
                                engine=inst.engine,
                                bass_nofuse=True,
                                ins=[],
                                outs=[],
                                sync_info=mybir.SyncInfo(on_wait=[w], on_update=[]),
                            )
                        )
                    si.on_wait = keep
                out.append(inst)
            if changed:
                try:
                    bb.instructions = out
                except Exception:
                    bb.instructions[:] = out


def build(T=1024, K=4096, O=4096, V=64, CV=1280, wq_split=4, split=True,
          noload=False):
    f32, f16 = mybir.dt.float32, mybir.dt.float16
    P = 128
    G = K // V                 # quant groups per row (64)
    KT = K // P                # contraction tiles (32)
    NT = T // P                # token tiles per core (8)
    OC = 512                   # oc chunk (psum bank width fp32)
    NOC = O // OC              # 8
    NPH = 2                    # W halves
    OCPH = NOC // NPH          # oc chunks per phase (4)
    GV = CV // V               # vector-side quant groups
    KQ = KT // wq_split        # W load quarters

    nc = bass.Bass()
    x = nc.dram_tensor("x", [T, K], f32, kind="ExternalInput")
    wt = nc.dram_tensor("wt", [NOC, P, KT * OC], f16, kind="ExternalInput")
    bvec = nc.dram_tensor("b", [O], f16, kind="ExternalInput")
    out = nc.dram_tensor("out", [T, O], f32, kind="ExternalOutput")
    xtd = nc.dram_tensor("xtd", [NT, P, KT * P], f16)  # x~^T spill

    mult = mybir.AluOpType.mult
    add = mybir.AluOpType.add
    sub = mybir.AluOpType.subtract
    amax_op = mybir.AluOpType.max

    with tile.TileContext(nc) as tc:
        with (
            tc.tile_pool(name="x", bufs=3) as pool_x,
            tc.tile_pool(name="xh", bufs=2) as pool_xh,
            tc.tile_pool(name="st", bufs=2) as pool_s,
            tc.tile_pool(name="xt", bufs=3) as pool_xt,
            tc.tile_pool(name="w", bufs=1) as pool_w,
            tc.tile_pool(name="bias", bufs=1) as pool_b,
            tc.tile_pool(name="o", bufs=3) as pool_o,
            tc.tile_pool(name="ps", bufs=1, space="PSUM") as pool_ps,
        ):
            def post_bias(ph):
                bt = pool_b.tile([P, OCPH, OC], f16, tag="b", name=f"b{ph}")
                bsl = bvec[ph * OCPH * OC : (ph + 1) * OCPH * OC]
                bb = bass.AP(
                    tensor=bsl.tensor, offset=bsl.offset, ap=[[0, P], *bsl.ap]
                )
                nc.sync.dma_start(out=bt.rearrange("p c o -> p (c o)"), in_=bb)
                return bt

            def post_w(ph, oc):
                wtile = pool_w.tile(
                    [P, KT, OC], f16, tag=f"w{oc}", name=f"w{ph}_{oc}"
                )
                g = ph * OCPH + oc
                for q in range(wq_split):
                    nc.sync.dma_start(
                        out=wtile[:, q * KQ : (q + 1) * KQ, :].rearrange(
                            "p k o -> p (k o)"
                        ),
                        in_=wt[g][:, q * KQ * OC : (q + 1) * KQ * OC],
                    )
                return wtile

            def emit_quant(t):
                # two independent half-chains (cols [0:K/2], [K/2:K]):
                # per-half stats + round let the first XBAR transpose (and
                # the first matmul of the tile) start before the second
                # half of x has even arrived.  round writes fp16 straight
                # into xh (integers <=127 are exact), dequant is in-place
                # on xh, so the x half-buffer frees at round time.
                H = K // 2
                GH = G // 2
                rows = slice(t * P, (t + 1) * P)
                xa = pool_x.tile([P, H], f32, tag="x", name=f"x{t}a")
                nc.gpsimd.dma_start(out=xa[:], in_=x[rows, :H])
                xb = pool_x.tile([P, H], f32, tag="x", name=f"x{t}b")
                nc.gpsimd.dma_start(out=xb[:], in_=x[rows, H:])
                xh_t = pool_xh.tile([P, K], f16, tag="xh", name=f"xh{t}")
                xhr = xh_t.rearrange("p (g v) -> p g v", v=V)
                xts_t = pool_xt.tile([P, KT, P], f16, tag="xt", name=f"xts{t}")
                amax = pool_s.tile([P, G], f32, tag="amax", name=f"amax{t}")
                delta = pool_s.tile([P, G], f32, tag="delta", name=f"delta{t}")
                recip = pool_s.tile([P, G], f32, tag="recip", name=f"recip{t}")

                def stats(g0, xr):
                    gs = slice(g0, g0 + GH)
                    nc.vector.tensor_reduce(
                        out=amax[:, gs], in_=xr, axis=mybir.AxisListType.X,
                        op=amax_op, apply_absolute_value=True,
                    )
                    nc.vector.tensor_scalar(
                        out=delta[:, gs], in0=amax[:, gs],
                        scalar1=QSCALE, scalar2=DELTA_MIN, op0=mult, op1=amax_op,
                    )
                    nc.vector.reciprocal(out=recip[:, gs], in_=delta[:, gs])

                def rmul(eng, xr, g0, l0, l1):   # x/delta (in-place on x half)
                    eng.tensor_tensor(
                        out=xr[:, l0:l1, :], in0=xr[:, l0:l1, :],
                        in1=recip[:, g0 + l0 : g0 + l1, None].to_broadcast(
                            (P, l1 - l0, V)), op=mult,
                    )

                def rnd(xt_):   # exact fp32 RNE via +/-MAGIC, in place
                    nc.vector.tensor_scalar(
                        out=xt_[:], in0=xt_[:],
                        scalar1=MAGIC, scalar2=MAGIC, op0=add, op1=sub,
                    )

                def dmul(eng, xr, g0, l0, l1):   # dequant, fp16 out into xh
                    eng.tensor_tensor(
                        out=xhr[:, g0 + l0 : g0 + l1, :], in0=xr[:, l0:l1, :],
                        in1=delta[:, g0 + l0 : g0 + l1, None].to_broadcast(
                            (P, l1 - l0, V)), op=mult,
                    )

                xra = xa.rearrange("p (g v) -> p g v", v=V)
                xrb = xb.rearrange("p (g v) -> p g v", v=V)
                # vector ops in dependency-monotone order
                # NB: Tile dependencies follow program order for in-place
                # updates — each half's gpsimd scale MUST be emitted before
                # that half's round.
                stats(0, xra)
                rmul(nc.vector, xra, 0, 0, GV)
                rmul(nc.gpsimd, xra, 0, GV, GH)
                stats(GH, xrb)
                rnd(xa)
                dmul(nc.vector, xra, 0, 0, GV)
                dmul(nc.gpsimd, xra, 0, GV, GH)
                rmul(nc.gpsimd, xrb, GH, 0, GH)
                rnd(xb)
                dmul(nc.gpsimd, xrb, GH, 0, GH)
                # per-half XBAR transpose -> [128k, KT, 128t], then spill
                nc.scalar.dma_start_transpose(xts_t[:, : KT // 2, :], xh_t[:, :H])
                nc.scalar.dma_start_transpose(xts_t[:, KT // 2 :, :], xh_t[:, H:])
                nc.sync.dma_start(
                    out=xtd[t], in_=xts_t.rearrange("p k q -> p (k q)")
                )
                return xts_t

            def evac(ph, t, oc, ps, bt):
                ot = pool_o.tile([P, OC], f32, tag="o", name=f"ot{ph}_{t}_{oc}")
                nc.vector.tensor_tensor(out=ot[:], in0=ps[:], in1=bt[:, oc, :], op=add)
                g = ph * OCPH + oc
                nc.sync.dma_start(
                    out=out[t * P : (t + 1) * P, g * OC : (g + 1) * OC], in_=ot[:]
                )

            def mm(ps, lhsT, rhs, start, stop, first):
                inst = nc.tensor.matmul(ps, lhsT, rhs, start=start, stop=stop)
                if noload and not first:
                    try:
                        inst.ldweights = False
                    except Exception:
                        try:
                            inst.ins.ldweights = False
                        except Exception:
                            pass

            def emit_mm(ph, t, xts_t, bt, ocs, mode, after_group=None):
                if mode == "oc":
                    for oc in ocs:
                        ps = pool_ps.tile([P, OC], f32, tag=f"ps{t % 2}_{oc}",
                                          name=f"ps{ph}_{t}_{oc}")
                        for kt in range(KT):
                            mm(ps[:], xts_t[:, kt, :], wcur[oc][:, kt, :],
                               kt == 0, kt == KT - 1, True)
                        evac(ph, t, oc, ps, bt)
                        if after_group is not None:
                            after_group(oc)
                else:
                    pss = {
                        oc: pool_ps.tile([P, OC], f32, tag=f"ps{t % 2}_{oc}",
                                         name=f"ps{ph}_{t}_{oc}")
                        for oc in ocs
                    }
                    for kt in range(KT):
                        for oc in ocs:
                            mm(pss[oc][:], xts_t[:, kt, :], wcur[oc][:, kt, :],
                               kt == 0, kt == KT - 1, oc == ocs[0])
                    for oc in ocs:
                        evac(ph, t, oc, pss[oc], bt)

            def reload(t):
                xr_t = pool_xt.tile([P, KT, P], f16, tag="xt", name=f"xtr{t}")
                nc.sync.dma_start(
                    out=xr_t.rearrange("p k q -> p (k q)"), in_=xtd[t]
                )
                return xr_t

            # ---- phase A (oc 0..3 resident), quant interleaved ----
            btA = post_bias(0)
            wcur = [post_w(0, oc) for oc in range(OCPH)]

            def swapcb(oc):
                wcur[oc] = post_w(1, oc)

            # Segments stagger t0's oc-groups around t1 so the W stream has
            # time to arrive (phase A) / swap in (phase B) without stalling
            # the PE, and the last phase-A tile runs oc-major so each W-half
            # buffer frees (and reloads with phase B) one group at a time.
            def segments(ph):
                segs = [(0, [0, 1], "oc", None), (1, ALL, "k", None),
                        (0, [2, 3], "oc", None)]
                for t in range(2, NT - 1):
                    segs.append((t, ALL, "k", None))
                segs.append((NT - 1, ALL, "oc" if ph == 0 else "k",
                             swapcb if ph == 0 else None))
                return segs

            ALL = list(range(OCPH))

            def reload(t):
                xr_t = pool_xt.tile([P, KT, P], f16, tag="xt", name=f"xtr{t}")
                nc.sync.dma_start(
                    out=xr_t.rearrange("p k q -> p (k q)"), in_=xtd[t]
                )
                return xr_t

            # ---- phase A (oc 0..3 resident), quant interleaved ----
            tiles = {}
            segsA = segments(0)
            qnext = 0

            def emit_q_upto(n):
                nonlocal qnext
                while qnext < min(n, NT):
                    tiles[qnext] = emit_quant(qnext)
                    qnext += 1

            emit_q_upto(2)
            rel = {}
            for si, (t, ocs, mode, cb) in enumerate(segsA):
                emit_mm(0, t, tiles[t], btA, ocs, mode, cb)
                emit_q_upto(3 + si)
                if t == NT - 3 and mode == "k":
                    rel[0] = reload(0)

            # ---- phase B (oc 4..7 resident), x~^T re-loaded from DRAM ----
            rel[1] = reload(1)
            btB = post_bias(1)
            rnext = 2

            for t, ocs, mode, cb in segments(1):
                emit_mm(1, t, rel[t], btB, ocs, mode, cb)
                if rnext < NT:
                    rel[rnext] = reload(rnext)
                    rnext += 1

    if split:
        _split_multiwait(nc)
    return nc


_CACHED = {}

# test-harness knobs (kernel() defaults are what the grader uses)
TRACE = False
LAST_RESULT = None
BUILD_KW = {}


def _get_nc(shape_key):
    if shape_key not in _CACHED:
        T, K, O = shape_key
        _CACHED[shape_key] = build(T=T, K=K, O=O, **BUILD_KW)
    return _CACHED[shape_key]


def pack_w(W: np.ndarray, OC: int = 512, P: int = 128) -> np.ndarray:
    # [out,in] -> W^T [in,out] fp16, packed [NOC, P, KT*OC] so each per-core
    # o-chunk W load is one fully contiguous DMA
    K, O = W.shape[1], W.shape[0]
    KT, NOC = K // P, O // OC
    wt = np.ascontiguousarray(W.T).astype(np.float16)         # [K, O]
    z = wt.reshape(KT, P, NOC, OC).transpose(2, 1, 0, 3)      # [NOC, P, KT, OC]
    return np.ascontiguousarray(z.reshape(NOC, P, KT * OC))


def kernel(x: np.ndarray, W: np.ndarray, b: np.ndarray) -> np.ndarray:
    global LAST_RESULT
    n, k = x.shape               # 8192, 4096
    o = W.shape[0]               # 4096
    assert n % N_CORES == 0
    tpc = n // N_CORES
    nc = _get_nc((tpc, k, o))

    wt = pack_w(W)
    b16 = np.ascontiguousarray(b.astype(np.float16))
    xs = np.ascontiguousarray(x.astype(np.float32)).reshape(N_CORES, tpc, k)
    in_maps = [{"x": xs[i], "wt": wt, "b": b16} for i in range(N_CORES)]
    res = run_bass_kernel_spmd(nc, in_maps, list(range(N_CORES)), trace=TRACE)
    LAST_RESULT = res
    return np.concatenate([res.results[i]["out"] for i in range(N_CORES)], axis=0)


# revision 14
# speedup vs baseline: 1.0295x; 1.0013x over previous
"""Trainium2 Bass kernel for group-quant (fake int8, V=64) + Linear.

reference math (per row of x):
    absmax over feature-groups of 64 -> delta = max(2*absmax/254, 1e-5)
    xq = clip(round(x/delta), -127, 127) * delta      (fake quant)
    out = xq @ W.T + b

Sharding: data-parallel on tokens across 8 cores (1024 rows each);
W (pre-transposed/packed fp16 on host) + b (fp16) replicated.

Device schedule per core (v2 — W-half resident, t-tile pipeline):
  quant runs per 128-token tile in natural layout, column-split across
  the vector and gpsimd engines (group absmax reduce, exact RNE round
  via the +/-1.5*2^23 trick, dequant to fp16), then ONE whole-tile
  SBUF->SBUF XBAR transpose (scalar queue, isolated from copy DMAs)
  produces x~^T [128k, KT, 128t].  Matmuls run in two phases, each with
  half of W^T (4 oc-chunks of 512, 16.8MB fp16) resident in SBUF:
  per t-tile, k-outer/oc-inner so 4 consecutive matmuls share one
  stationary (LDWEIGHTS amortized 4x), accumulating into 4 PSUM banks,
  ping-ponged with the other 4 across t-tiles.  First/last tiles of
  each phase run oc-major to stagger W DMA arrival/swap.  x~^T tiles
  are spilled to DRAM in phase A and re-loaded in phase B.  PSUM is
  evacuated fused with the bias add on vector; output DMAs ride the
  vector queue so they never head-of-line-block W loads (sync queue).
"""

import numpy as np

import concourse.bass as bass
import concourse.mybir as mybir
import concourse.tile as tile
from concourse.bass_utils import run_bass_kernel_spmd

N_CORES = 8
MAGIC = 1.5 * 2.0**23      # fp32 round-to-nearest-even constant
QSCALE = 1.0 / 127.0       # 2/(qmax-qmin) with qmax=127, qmin=-127
DELTA_MIN = 1e-5


def _split_multiwait(nc):
    """This walrus build allows at most ONE sync wait per instruction
    ("Too many sync wait commands", CoreV3GenImpl setupSyncWait) and none
    on Drain. Tile freely attaches several waits to one instruction, so
    post-process: move excess waits onto single-wait NoOps inserted just
    before the instruction on the same engine queue (semantics identical —
    the queue stalls at the nop instead of at the instruction)."""
    nid = 0
    for fn in nc.m.functions:
        for bb in fn.blocks:
            insts = list(bb.instructions)
            out = []
            changed = False
            for inst in insts:
                si = inst.sync_info
                waits = list(si.on_wait) if si is not None and si.on_wait else []
                limit = 0 if type(inst).__name__ == "InstDrain" else 1
                if len(waits) > limit:
                    changed = True
                    keep = waits[len(waits) - limit :] if limit else []
                    for w in waits[: len(waits) - limit]:
                        nid += 1
                        out.append(
                            mybir.InstNoOp(
                                name=f"WSPLIT-{nid}",
                                engine=inst.engine,
                                bass_nofuse=True,
                                ins=[],
                                outs=[],
                                sync_info=mybir.SyncInfo(on_wait=[w], on_update=[]),
                            )
                        )
                    si.on_wait = keep
                out.append(inst)
            if changed:
                try:
                    bb.instructions = out
                except Exception:
                    bb.instructions[:] = out


def build(T=1024, K=4096, O=4096, V=64, CV=1280, wq_split=4, split=True,
          noload=False):
    f32, f16 = mybir.dt.float32, mybir.dt.float16
    P = 128
    G = K // V                 # quant groups per row (64)
    KT = K // P                # contraction tiles (32)
    NT = T // P                # token tiles per core (8)
    OC = 512                   # oc chunk (psum bank width fp32)
    NOC = O // OC              # 8
    NPH = 2                    # W halves
    OCPH = NOC // NPH          # oc chunks per phase (4)
    GV = CV // V               # vector-side quant groups
    KQ = KT // wq_split        # W load quarters

    nc = bass.Bass()
    x = nc.dram_tensor("x", [T, K], f32, kind="ExternalInput")
    wt = nc.dram_tensor("wt", [NOC, P, KT * OC], f16, kind="ExternalInput")
    bvec = nc.dram_tensor("b", [O], f16, kind="ExternalInput")
    out = nc.dram_tensor("out", [T, O], f32, kind="ExternalOutput")
    xtd = nc.dram_tensor("xtd", [NT, P, KT * P], f16)  # x~^T spill

    mult = mybir.AluOpType.mult
    add = mybir.AluOpType.add
    sub = mybir.AluOpType.subtract
    amax_op = mybir.AluOpType.max

    with tile.TileContext(nc) as tc:
        with (
            tc.tile_pool(name="x", bufs=3) as pool_x,
            tc.tile_pool(name="xh", bufs=2) as pool_xh,
            tc.tile_pool(name="st", bufs=2) as pool_s,
            tc.tile_pool(name="xt", bufs=3) as pool_xt,
            tc.tile_pool(name="w", bufs=1) as pool_w,
            tc.tile_pool(name="bias", bufs=1) as pool_b,
            tc.tile_pool(name="o", bufs=3) as pool_o,
            tc.tile_pool(name="ps", bufs=1, space="PSUM") as pool_ps,
        ):
            def post_bias(ph):
                bt = pool_b.tile([P, OCPH, OC], f16, tag="b", name=f"b{ph}")
                bsl = bvec[ph * OCPH * OC : (ph + 1) * OCPH * OC]
                bb = bass.AP(
                    tensor=bsl.tensor, offset=bsl.offset, ap=[[0, P], *bsl.ap]
                )
                nc.sync.dma_start(out=bt.rearrange("p c o -> p (c o)"), in_=bb)
                return bt

            def post_w(ph, oc):
                wtile = pool_w.tile(
                    [P, KT, OC], f16, tag=f"w{oc}", name=f"w{ph}_{oc}"
                )
                g = ph * OCPH + oc
                for q in range(wq_split):
                    nc.sync.dma_start(
                        out=wtile[:, q * KQ : (q + 1) * KQ, :].rearrange(
                            "p k o -> p (k o)"
                        ),
                        in_=wt[g][:, q * KQ * OC : (q + 1) * KQ * OC],
                    )
                return wtile

            def emit_quant(t):
                # two independent half-chains (cols [0:K/2], [K/2:K]):
                # per-half stats + round let the first XBAR transpose (and
                # the first matmul of the tile) start before the second
                # half of x has even arrived.  round writes fp16 straight
                # into xh (integers <=127 are exact), dequant is in-place
                # on xh, so the x half-buffer frees at round time.
                H = K // 2
                GH = G // 2
                rows = slice(t * P, (t + 1) * P)
                xa = pool_x.tile([P, H], f32, tag="x", name=f"x{t}a")
                nc.gpsimd.dma_start(out=xa[:], in_=x[rows, :H])
                xb = pool_x.tile([P, H], f32, tag="x", name=f"x{t}b")
                nc.gpsimd.dma_start(out=xb[:], in_=x[rows, H:])
                xh_t = pool_xh.tile([P, K], f16, tag="xh", name=f"xh{t}")
                xhr = xh_t.rearrange("p (g v) -> p g v", v=V)
                xts_t = pool_xt.tile([P, KT, P], f16, tag="xt", name=f"xts{t}")
                amax = pool_s.tile([P, G], f32, tag="amax", name=f"amax{t}")
                delta = pool_s.tile([P, G], f32, tag="delta", name=f"delta{t}")
                recip = pool_s.tile([P, G], f32, tag="recip", name=f"recip{t}")

                def stats(g0, xr):
                    gs = slice(g0, g0 + GH)
                    nc.vector.tensor_reduce(
                        out=amax[:, gs], in_=xr, axis=mybir.AxisListType.X,
                        op=amax_op, apply_absolute_value=True,
                    )
                    nc.vector.tensor_scalar(
                        out=delta[:, gs], in0=amax[:, gs],
                        scalar1=QSCALE, scalar2=DELTA_MIN, op0=mult, op1=amax_op,
                    )
                    nc.vector.reciprocal(out=recip[:, gs], in_=delta[:, gs])

                def rmul(eng, xr, g0, l0, l1):   # x/delta (in-place on x half)
                    eng.tensor_tensor(
                        out=xr[:, l0:l1, :], in0=xr[:, l0:l1, :],
                        in1=recip[:, g0 + l0 : g0 + l1, None].to_broadcast(
                            (P, l1 - l0, V)), op=mult,
                    )

                def rnd(xt_):   # exact fp32 RNE via +/-MAGIC, in place
                    nc.vector.tensor_scalar(
                        out=xt_[:], in0=xt_[:],
                        scalar1=MAGIC, scalar2=MAGIC, op0=add, op1=sub,
                    )

                def dmul(eng, xr, g0, l0, l1):   # dequant, fp16 out into xh
                    eng.tensor_tensor(
                        out=xhr[:, g0 + l0 : g0 + l1, :], in0=xr[:, l0:l1, :],
                        in1=delta[:, g0 + l0 : g0 + l1, None].to_broadcast(
                            (P, l1 - l0, V)), op=mult,
                    )

                xra = xa.rearrange("p (g v) -> p g v", v=V)
                xrb = xb.rearrange("p (g v) -> p g v", v=V)
                # vector ops in dependency-monotone order
                # NB: Tile dependencies follow program order for in-place
                # updates — each half's gpsimd scale MUST be emitted before
                # that half's round.
                stats(0, xra)
                rmul(nc.vector, xra, 0, 0, GV)
                rmul(nc.gpsimd, xra, 0, GV, GH)
                stats(GH, xrb)
                rnd(xa)
                dmul(nc.vector, xra, 0, 0, GV)
                dmul(nc.gpsimd, xra, 0, GV, GH)
                rmul(nc.gpsimd, xrb, GH, 0, GH)
                rnd(xb)
                dmul(nc.gpsimd, xrb, GH, 0, GH)
                # per-half XBAR transpose -> [128k, KT, 128t], then spill
                nc.scalar.dma_start_transpose(xts_t[:, : KT // 2, :], xh_t[:, :H])
                nc.scalar.dma_start_transpose(xts_t[:, KT // 2 :, :], xh_t[:, H:])
                nc.sync.dma_start(
                    out=xtd[t], in_=xts_t.rearrange("p k q -> p (k q)")
                )
                return xts_t

            def evac(ph, t, oc, ps, bt):
                ot = pool_o.tile([P, OC], f32, tag="o", name=f"ot{ph}_{t}_{oc}")
                nc.vector.tensor_tensor(out=ot[:], in0=ps[:], in1=bt[:, oc, :], op=add)
                g = ph * OCPH + oc
                nc.sync.dma_start(
                    out=out[t * P : (t + 1) * P, g * OC : (g + 1) * OC], in_=ot[:]
                )

            def mm(ps, lhsT, rhs, start, stop, first):
                inst = nc.tensor.matmul(ps, lhsT, rhs, start=start, stop=stop)
                if noload and not first:
                    try:
                        inst.ldweights = False
                    except Exception:
                        try:
                            inst.ins.ldweights = False
                        except Exception:
                            pass

            def emit_mm(ph, t, xts_t, bt, ocs, mode, after_group=None):
                if mode == "oc":
                    for oc in ocs:
                        ps = pool_ps.tile([P, OC], f32, tag=f"ps{t % 2}_{oc}",
                                          name=f"ps{ph}_{t}_{oc}")
                        for kt in range(KT):
                            mm(ps[:], xts_t[:, kt, :], wcur[oc][:, kt, :],
                               kt == 0, kt == KT - 1, True)
                        evac(ph, t, oc, ps, bt)
                        if after_group is not None:
                            after_group(oc)
                else:
                    pss = {
                        oc: pool_ps.tile([P, OC], f32, tag=f"ps{t % 2}_{oc}",
                                         name=f"ps{ph}_{t}_{oc}")
                        for oc in ocs
                    }
                    for kt in range(KT):
                        for oc in ocs:
                            mm(pss[oc][:], xts_t[:, kt, :], wcur[oc][:, kt, :],
                               kt == 0, kt == KT - 1, oc == ocs[0])
                    for oc in ocs:
                        evac(ph, t, oc, pss[oc], bt)

            def reload(t):
                xr_t = pool_xt.tile([P, KT, P], f16, tag="xt", name=f"xtr{t}")
                nc.sync.dma_start(
                    out=xr_t.rearrange("p k q -> p (k q)"), in_=xtd[t]
                )
                return xr_t

            # ---- phase A (oc 0..3 resident), quant interleaved ----
            btA = post_bias(0)
            wcur = [post_w(0, oc) for oc in range(OCPH)]

            def swapcb(oc):
                wcur[oc] = post_w(1, oc)

            # Segments stagger t0's oc-groups around t1 so the W stream has
            # time to arrive (phase A) / swap in (phase B) without stalling
            # the PE, and the last phase-A tile runs oc-major so each W-half
            # buffer frees (and reloads with phase B) one group at a time.
            def segments(ph):
                # oc-pairs spread over the first three tiles: the W half
                # streams in at only ~160-220 GB/s against x-load + XBAR
                # contention, so demand for its second pair must not start
                # before ~90us (phase A) / mid-swap (phase B).
                segs = [(0, [0, 1], "oc", None), (1, [0, 1], "k", None),
                        (2, [0, 1], "k", None), (0, [2, 3], "oc", None),
                        (1, [2, 3], "k", None), (2, [2, 3], "k", None)]
                for t in range(3, NT - 1):
                    segs.append((t, ALL, "k", None))
                segs.append((NT - 1, ALL, "oc" if ph == 0 else "k",
                             swapcb if ph == 0 else None))
                return segs

            ALL = list(range(OCPH))

            def reload(t):
                xr_t = pool_xt.tile([P, KT, P], f16, tag="xt", name=f"xtr{t}")
                nc.sync.dma_start(
                    out=xr_t.rearrange("p k q -> p (k q)"), in_=xtd[t]
                )
                return xr_t

            # ---- phase A (oc 0..3 resident), quant interleaved ----
            tiles = {}
            segsA = segments(0)
            qnext = 0

            def emit_q_upto(n):
                nonlocal qnext
                while qnext < min(n, NT):
                    tiles[qnext] = emit_quant(qnext)
                    qnext += 1

            emit_q_upto(2)
            rel = {}
            for si, (t, ocs, mode, cb) in enumerate(segsA):
                emit_mm(0, t, tiles[t], btA, ocs, mode, cb)
                emit_q_upto(3 + si)
                if t == NT - 3 and mode == "k":
                    rel[0] = reload(0)

            # ---- phase B (oc 4..7 resident), x~^T re-loaded from DRAM ----
            rel[1] = reload(1)
            btB = post_bias(1)
            rnext = 2

            for t, ocs, mode, cb in segments(1):
                emit_mm(1, t, rel[t], btB, ocs, mode, cb)
                if rnext < NT:
                    rel[rnext] = reload(rnext)
                    rnext += 1

    if split:
        _split_multiwait(nc)
    return nc


_CACHED = {}

# test-harness knobs (kernel() defaults are what the grader uses)
TRACE = False
LAST_RESULT = None
BUILD_KW = {}


def _get_nc(shape_key):
    if shape_key not in _CACHED:
        T, K, O = shape_key
        _CACHED[shape_key] = build(T=T, K=K, O=O, **BUILD_KW)
    return _CACHED[shape_key]


def pack_w(W: np.ndarray, OC: int = 512, P: int = 128) -> np.ndarray:
    # [out,in] -> W^T [in,out] fp16, packed [NOC, P, KT*OC] so each per-core
    # o-chunk W load is one fully contiguous DMA
    K, O = W.shape[1], W.shape[0]
    KT, NOC = K // P, O // OC
    wt = np.ascontiguousarray(W.T).astype(np.float16)         # [K, O]
    z = wt.reshape(KT, P, NOC, OC).transpose(2, 1, 0, 3)      # [NOC, P, KT, OC]
    return np.ascontiguousarray(z.reshape(NOC, P, KT * OC))


def kernel(x: np.ndarray, W: np.ndarray, b: np.ndarray) -> np.ndarray:
    global LAST_RESULT
    n, k = x.shape               # 8192, 4096
    o = W.shape[0]               # 4096
    assert n % N_CORES == 0
    tpc = n // N_CORES
    nc = _get_nc((tpc, k, o))

    wt = pack_w(W)
    b16 = np.ascontiguousarray(b.astype(np.float16))
    xs = np.ascontiguousarray(x.astype(np.float32)).reshape(N_CORES, tpc, k)
    in_maps = [{"x": xs[i], "wt": wt, "b": b16} for i in range(N_CORES)]
    res = run_bass_kernel_spmd(nc, in_maps, list(range(N_CORES)), trace=TRACE)
    LAST_RESULT = res
    return np.concatenate([res.results[i]["out"] for i in range(N_CORES)], axis=0)


# revision 15
# speedup vs baseline: 1.0316x; 1.0021x over previous
"""Trainium2 Bass kernel for group-quant (fake int8, V=64) + Linear.

reference math (per row of x):
    absmax over feature-groups of 64 -> delta = max(2*absmax/254, 1e-5)
    xq = clip(round(x/delta), -127, 127) * delta      (fake quant)
    out = xq @ W.T + b

Sharding: data-parallel on tokens across 8 cores (1024 rows each);
W (pre-transposed/packed fp16 on host) + b (fp16) replicated.

Device schedule per core (v2 — W-half resident, t-tile pipeline):
  quant runs per 128-token tile in natural layout, column-split across
  the vector and gpsimd engines (group absmax reduce, exact RNE round
  via the +/-1.5*2^23 trick, dequant to fp16), then ONE whole-tile
  SBUF->SBUF XBAR transpose (scalar queue, isolated from copy DMAs)
  produces x~^T [128k, KT, 128t].  Matmuls run in two phases, each with
  half of W^T (4 oc-chunks of 512, 16.8MB fp16) resident in SBUF:
  per t-tile, k-outer/oc-inner so 4 consecutive matmuls share one
  stationary (LDWEIGHTS amortized 4x), accumulating into 4 PSUM banks,
  ping-ponged with the other 4 across t-tiles.  First/last tiles of
  each phase run oc-major to stagger W DMA arrival/swap.  x~^T tiles
  are spilled to DRAM in phase A and re-loaded in phase B.  PSUM is
  evacuated fused with the bias add on vector; output DMAs ride the
  vector queue so they never head-of-line-block W loads (sync queue).
"""

import numpy as np

import concourse.bass as bass
import concourse.mybir as mybir
import concourse.tile as tile
from concourse.bass_utils import run_bass_kernel_spmd

N_CORES = 8
MAGIC = 1.5 * 2.0**23      # fp32 round-to-nearest-even constant
QSCALE = 1.0 / 127.0       # 2/(qmax-qmin) with qmax=127, qmin=-127
DELTA_MIN = 1e-5


def _split_multiwait(nc):
    """This walrus build allows at most ONE sync wait per instruction
    ("Too many sync wait commands", CoreV3GenImpl setupSyncWait) and none
    on Drain. Tile freely attaches several waits to one instruction, so
    post-process: move excess waits onto single-wait NoOps inserted just
    before the instruction on the same engine queue (semantics identical —
    the queue stalls at the nop instead of at the instruction)."""
    nid = 0
    for fn in nc.m.functions:
        for bb in fn.blocks:
            insts = list(bb.instructions)
            out = []
            changed = False
            for inst in insts:
                si = inst.sync_info
                waits = list(si.on_wait) if si is not None and si.on_wait else []
                limit = 0 if type(inst).__name__ == "InstDrain" else 1
                if len(waits) > limit:
                    changed = True
                    keep = waits[len(waits) - limit :] if limit else []
                    for w in waits[: len(waits) - limit]:
                        nid += 1
                        out.append(
                            mybir.InstNoOp(
                                name=f"WSPLIT-{nid}",
                                engine=inst.engine,
                                bass_nofuse=True,
                                ins=[],
                                outs=[],
                                sync_info=mybir.SyncInfo(on_wait=[w], on_update=[]),
                            )
                        )
                    si.on_wait = keep
                out.append(inst)
            if changed:
                try:
                    bb.instructions = out
                except Exception:
                    bb.instructions[:] = out


def build(T=1024, K=4096, O=4096, V=64, CV=1280, wq_split=4, split=True,
          noload=False):
    f32, f16 = mybir.dt.float32, mybir.dt.float16
    P = 128
    G = K // V                 # quant groups per row (64)
    KT = K // P                # contraction tiles (32)
    NT = T // P                # token tiles per core (8)
    OC = 512                   # oc chunk (psum bank width fp32)
    NOC = O // OC              # 8
    NPH = 2                    # W halves
    OCPH = NOC // NPH          # oc chunks per phase (4)
    GV = CV // V               # vector-side quant groups
    KQ = KT // wq_split        # W load quarters

    nc = bass.Bass()
    x = nc.dram_tensor("x", [T, K], f32, kind="ExternalInput")
    wt = nc.dram_tensor("wt", [NOC, P, KT * OC], f16, kind="ExternalInput")
    out = nc.dram_tensor("out", [T, O], f32, kind="ExternalOutput")
    xtd = nc.dram_tensor("xtd", [NT, P, KT * P], f16)  # x~^T spill

    mult = mybir.AluOpType.mult
    add = mybir.AluOpType.add
    sub = mybir.AluOpType.subtract
    amax_op = mybir.AluOpType.max

    with tile.TileContext(nc) as tc:
        with (
            tc.tile_pool(name="x", bufs=3) as pool_x,
            tc.tile_pool(name="xh", bufs=2) as pool_xh,
            tc.tile_pool(name="st", bufs=2) as pool_s,
            tc.tile_pool(name="xt", bufs=3) as pool_xt,
            tc.tile_pool(name="w", bufs=1) as pool_w,
            tc.tile_pool(name="o", bufs=4) as pool_o,
            tc.tile_pool(name="ps", bufs=1, space="PSUM") as pool_ps,
        ):
            def post_w(ph, oc):
                wtile = pool_w.tile(
                    [P, KT, OC], f16, tag=f"w{oc}", name=f"w{ph}_{oc}"
                )
                g = ph * OCPH + oc
                for q in range(wq_split):
                    nc.sync.dma_start(
                        out=wtile[:, q * KQ : (q + 1) * KQ, :].rearrange(
                            "p k o -> p (k o)"
                        ),
                        in_=wt[g][:, q * KQ * OC : (q + 1) * KQ * OC],
                    )
                return wtile

            def emit_quant(t):
                # two independent half-chains (cols [0:K/2], [K/2:K]):
                # per-half stats + round let the first XBAR transpose (and
                # the first matmul of the tile) start before the second
                # half of x has even arrived.  round writes fp16 straight
                # into xh (integers <=127 are exact), dequant is in-place
                # on xh, so the x half-buffer frees at round time.
                H = K // 2
                GH = G // 2
                rows = slice(t * P, (t + 1) * P)
                xa = pool_x.tile([P, H], f32, tag="x", name=f"x{t}a")
                nc.gpsimd.dma_start(out=xa[:], in_=x[rows, :H])
                xb = pool_x.tile([P, H], f32, tag="x", name=f"x{t}b")
                nc.gpsimd.dma_start(out=xb[:], in_=x[rows, H:])
                xh_t = pool_xh.tile([P, K], f16, tag="xh", name=f"xh{t}")
                xhr = xh_t.rearrange("p (g v) -> p g v", v=V)
                xts_t = pool_xt.tile([P, KT, P], f16, tag="xt", name=f"xts{t}")
                amax = pool_s.tile([P, G], f32, tag="amax", name=f"amax{t}")
                delta = pool_s.tile([P, G], f32, tag="delta", name=f"delta{t}")
                recip = pool_s.tile([P, G], f32, tag="recip", name=f"recip{t}")

                def stats(g0, xr):
                    gs = slice(g0, g0 + GH)
                    nc.vector.tensor_reduce(
                        out=amax[:, gs], in_=xr, axis=mybir.AxisListType.X,
                        op=amax_op, apply_absolute_value=True,
                    )
                    nc.vector.tensor_scalar(
                        out=delta[:, gs], in0=amax[:, gs],
                        scalar1=QSCALE, scalar2=DELTA_MIN, op0=mult, op1=amax_op,
                    )
                    nc.vector.reciprocal(out=recip[:, gs], in_=delta[:, gs])

                def rmul(eng, xr, g0, l0, l1):   # x/delta (in-place on x half)
                    eng.tensor_tensor(
                        out=xr[:, l0:l1, :], in0=xr[:, l0:l1, :],
                        in1=recip[:, g0 + l0 : g0 + l1, None].to_broadcast(
                            (P, l1 - l0, V)), op=mult,
                    )

                def rnd(xt_):   # exact fp32 RNE via +/-MAGIC, in place
                    nc.vector.tensor_scalar(
                        out=xt_[:], in0=xt_[:],
                        scalar1=MAGIC, scalar2=MAGIC, op0=add, op1=sub,
                    )

                def dmul(eng, xr, g0, l0, l1):   # dequant, fp16 out into xh
                    eng.tensor_tensor(
                        out=xhr[:, g0 + l0 : g0 + l1, :], in0=xr[:, l0:l1, :],
                        in1=delta[:, g0 + l0 : g0 + l1, None].to_broadcast(
                            (P, l1 - l0, V)), op=mult,
                    )

                xra = xa.rearrange("p (g v) -> p g v", v=V)
                xrb = xb.rearrange("p (g v) -> p g v", v=V)
                # vector ops in dependency-monotone order
                # NB: Tile dependencies follow program order for in-place
                # updates — each half's gpsimd scale MUST be emitted before
                # that half's round.
                stats(0, xra)
                rmul(nc.vector, xra, 0, 0, GV)
                rmul(nc.gpsimd, xra, 0, GV, GH)
                stats(GH, xrb)
                rnd(xa)
                dmul(nc.vector, xra, 0, 0, GV)
                dmul(nc.gpsimd, xra, 0, GV, GH)
                rmul(nc.gpsimd, xrb, GH, 0, GH)
                rnd(xb)
                dmul(nc.gpsimd, xrb, GH, 0, GH)
                # per-half XBAR transpose -> [128k, KT, 128t]
                nc.scalar.dma_start_transpose(xts_t[:, : KT // 2, :], xh_t[:, :H])
                nc.scalar.dma_start_transpose(xts_t[:, KT // 2 :, :], xh_t[:, H:])
                return xts_t

            def evac(ph, t, oc, ps):
                # plain PSUM->SBUF copy on the ACT engine: keeps the vector
                # queue pure quant (bias is added on the host)
                ot = pool_o.tile([P, OC], f32, tag="o", name=f"ot{ph}_{t}_{oc}")
                nc.scalar.activation(
                    out=ot[:], in_=ps[:], func=mybir.ActivationFunctionType.Copy
                )
                g = ph * OCPH + oc
                nc.sync.dma_start(
                    out=out[t * P : (t + 1) * P, g * OC : (g + 1) * OC], in_=ot[:]
                )

            def mm(ps, lhsT, rhs, start, stop, first):
                inst = nc.tensor.matmul(ps, lhsT, rhs, start=start, stop=stop)
                if noload and not first:
                    try:
                        inst.ldweights = False
                    except Exception:
                        try:
                            inst.ins.ldweights = False
                        except Exception:
                            pass

            def emit_mm(ph, t, xts_t, ocs, mode, after_group=None):
                if mode == "oc":
                    for oc in ocs:
                        ps = pool_ps.tile([P, OC], f32, tag=f"ps{t % 2}_{oc}",
                                          name=f"ps{ph}_{t}_{oc}")
                        for kt in range(KT):
                            mm(ps[:], xts_t[:, kt, :], wcur[oc][:, kt, :],
                               kt == 0, kt == KT - 1, True)
                        evac(ph, t, oc, ps)
                        if after_group is not None:
                            after_group(oc)
                else:
                    pss = {
                        oc: pool_ps.tile([P, OC], f32, tag=f"ps{t % 2}_{oc}",
                                         name=f"ps{ph}_{t}_{oc}")
                        for oc in ocs
                    }
                    for kt in range(KT):
                        for oc in ocs:
                            mm(pss[oc][:], xts_t[:, kt, :], wcur[oc][:, kt, :],
                               kt == 0, kt == KT - 1, oc == ocs[0])
                    for oc in ocs:
                        evac(ph, t, oc, pss[oc])

            # ---- phase A (oc 0..3 resident), quant interleaved ----
            wcur = [post_w(0, oc) for oc in range(OCPH)]

            def swapcb(oc):
                wcur[oc] = post_w(1, oc)

            # Segments stagger t0's oc-groups around t1 so the W stream has
            # time to arrive (phase A) / swap in (phase B) without stalling
            # the PE, and the last phase-A tile runs oc-major so each W-half
            # buffer frees (and reloads with phase B) one group at a time.
            def segments(ph):
                # oc-pairs spread over the first three tiles: the W half
                # streams in at only ~160-220 GB/s against x-load + XBAR
                # contention, so demand for its second pair must not start
                # before ~90us (phase A) / mid-swap (phase B).
                segs = [(0, [0, 1], "oc", None), (1, [0, 1], "k", None),
                        (2, [0, 1], "k", None), (0, [2, 3], "oc", None),
                        (1, [2, 3], "k", None), (2, [2, 3], "k", None)]
                for t in range(3, NT - 1):
                    segs.append((t, ALL, "k", None))
                segs.append((NT - 1, ALL, "oc" if ph == 0 else "k",
                             swapcb if ph == 0 else None))
                return segs

            ALL = list(range(OCPH))

            def reload(t):
                xr_t = pool_xt.tile([P, KT, P], f16, tag="xt", name=f"xtr{t}")
                nc.sync.dma_start(
                    out=xr_t.rearrange("p k q -> p (k q)"), in_=xtd[t]
                )
                return xr_t

            # ---- phase A (oc 0..3 resident), quant interleaved ----
            tiles = {}
            segsA = segments(0)
            qnext = 0

            def emit_q_upto(n):
                nonlocal qnext
                while qnext < min(n, NT):
                    tiles[qnext] = emit_quant(qnext)
                    qnext += 1

            def spill(t):
                nc.sync.dma_start(
                    out=xtd[t], in_=tiles[t].rearrange("p k q -> p (k q)")
                )

            emit_q_upto(4)
            rel = {}
            for si, (t, ocs, mode, cb) in enumerate(segsA):
                emit_mm(0, t, tiles[t], ocs, mode, cb)
                emit_q_upto(5 + si)
                # spills deferred off the bandwidth-critical early window
                if 2 <= si <= 9:
                    spill(si - 2)
                if t == NT - 3 and mode == "k":
                    rel[0] = reload(0)

            # ---- phase B (oc 4..7 resident), x~^T re-loaded from DRAM ----
            rel[1] = reload(1)
            rnext = 2

            for t, ocs, mode, cb in segments(1):
                emit_mm(1, t, rel[t], ocs, mode, cb)
                if rnext < NT:
                    rel[rnext] = reload(rnext)
                    rnext += 1

    if split:
        _split_multiwait(nc)
    return nc


_CACHED = {}

# test-harness knobs (kernel() defaults are what the grader uses)
TRACE = False
LAST_RESULT = None
BUILD_KW = {}


def _get_nc(shape_key):
    if shape_key not in _CACHED:
        T, K, O = shape_key
        _CACHED[shape_key] = build(T=T, K=K, O=O, **BUILD_KW)
    return _CACHED[shape_key]


def pack_w(W: np.ndarray, OC: int = 512, P: int = 128) -> np.ndarray:
    # [out,in] -> W^T [in,out] fp16, packed [NOC, P, KT*OC] so each per-core
    # o-chunk W load is one fully contiguous DMA
    K, O = W.shape[1], W.shape[0]
    KT, NOC = K // P, O // OC
    wt = np.ascontiguousarray(W.T).astype(np.float16)         # [K, O]
    z = wt.reshape(KT, P, NOC, OC).transpose(2, 1, 0, 3)      # [NOC, P, KT, OC]
    return np.ascontiguousarray(z.reshape(NOC, P, KT * OC))


def kernel(x: np.ndarray, W: np.ndarray, b: np.ndarray) -> np.ndarray:
    global LAST_RESULT
    n, k = x.shape               # 8192, 4096
    o = W.shape[0]               # 4096
    assert n % N_CORES == 0
    tpc = n // N_CORES
    nc = _get_nc((tpc, k, o))

    wt = pack_w(W)
    xs = np.ascontiguousarray(x.astype(np.float32)).reshape(N_CORES, tpc, k)
    in_maps = [{"x": xs[i], "wt": wt} for i in range(N_CORES)]
    res = run_bass_kernel_spmd(nc, in_maps, list(range(N_CORES)), trace=TRACE)
    LAST_RESULT = res
    full = np.concatenate(
        [res.results[i]["out"] for i in range(N_CORES)], axis=0
    )
    full += b.astype(np.float32)[None, :]
    return full
